# revision 9
# baseline (speedup 1.0000x reference)
"""MinRNN Trainium2 kernel — quasi-DEER fixed-point iteration, v6.

Model (per batch row):
    z_t = tanh(x_t @ W_in^T + b_in)
    u_t = sigmoid(s_{t-1} @ W_rec^T + z_t @ U_z^T + b_u)
    s_t = u_t * s_{t-1} + (1 - u_t) * z_t

Reformulated on the deviation m = s - z (with the convention z_{-1} = 0,
so m_{-1} = 0):

    pre_t = W_rec m_{t-1} + ct_t,   ct_t = U_z z_t + W_rec z_{t-1} + b_u
    u_t   = sigmoid(pre_t)
    m_t   = (dz_t + m_{t-1}) * u_t,  dz_t = z_{t-1} - z_t
    s_t   = z_t + m_t

Solved by fixed-point sweeps (quasi-DEER): freeze u from the previous
iterate, then the m-recurrence is solved EXACTLY by the HW linear-scan
instruction (tensor_tensor_scan computes state=(d0+state)*d1 in fp32).
K=4 effective sweeps (first is GEMM-free since m^0=0) converge to
rel err ~6.8e-3, 3x inside the 2e-2 tolerance.

v6 structural changes over v5 (225.4us):
  - scan groups sb=1024 steps (sp=4 tiles): 16 full-width scans per
    sweep instead of 32 — halves the ~0.7us/instr DVE fixed cost.
  - b-major (t innermost) layout for z/dz/ct/m/u: every scan operand
    is stride-1 (stride-2 APs ran the scans at ~1.5 cycles/elem).
  - psum tiles [128, 1024] (2 banks): ACT instructions run 1024 wide,
    halving the ~0.29us/instr ACT fixed cost (264 -> 132 ACTIVATEs).
  - dz on the Vector engine instead of GpSimd: the GpSimd SBUF port is
    an exclusive lock shared with DVE, so pool work stalled the scans.
  - u stored in per-cm [128, 2048] tiles (6 bufs) to keep SBUF at the
    v5 footprint.
Data-parallel over batch: 8 cores x 2 rows; fp16 GEMMs, fp32 psum.
GEMM moving columns are ordered [b][t] to match.
"""

import numpy as np
import ml_dtypes

import concourse.bass as bass
import concourse.mybir as mybir
import concourse.tile as tile
import concourse.bacc as bacc
from concourse import bass_utils

AF = mybir.ActivationFunctionType
OP = mybir.AluOpType

B, T, I, H = 16, 2048, 512, 512
N_CORES = 8
BL = B // N_CORES          # batch rows per core (2)
KC = I // 128              # input-dim chunks (4)
HC = H // 128              # hidden-dim chunks (4)
TB = 256                   # t-steps per GEMM tile (512 moving columns)
K_SWEEPS = 4               # effective sweeps (first one is GEMM-free)

f32 = mybir.dt.float32
f16 = mybir.dt.float16


def build(t_steps: int = T, tb: int = TB, sweeps: int = K_SWEEPS,
          compile: bool = True):
    tb = min(tb, t_steps)
    assert t_steps % tb == 0
    T1 = t_steps + 1

    nc = bacc.Bacc("TRN2", target_bir_lowering=False, debug=False)

    # x pre-tiled on the host: [tile, 128, (k t b)] so each tile DMA is a
    # fully contiguous [128, KC*tb*BL] block (4KB runs per partition)
    xT = nc.dram_tensor("xT", [t_steps // tb, 128, KC * tb * BL], f16,
                        kind="ExternalInput")
    winT = nc.dram_tensor("winT", [KC, 128, H], f16, kind="ExternalInput")
    wrecT = nc.dram_tensor("wrecT", [HC, 128, H], f16, kind="ExternalInput")
    uzT = nc.dram_tensor("uzT", [HC, 128, H], f16, kind="ExternalInput")
    binNeg = nc.dram_tensor("binNeg", [HC, 128], f32, kind="ExternalInput")
    bu2 = nc.dram_tensor("bu2", [HC, 128], f32, kind="ExternalInput")
    ident = nc.dram_tensor("ident", [128, 128], f16, kind="ExternalInput")
    mOut = nc.dram_tensor("mOut", [128, KC, BL, T1], f16, kind="ExternalOutput")
    zOut = nc.dram_tensor("zOut", [128, HC, BL, T1], f16, kind="ExternalOutput")

    with tile.TileContext(nc) as tc:
        _body(tc, nc, xT, winT, wrecT, uzT, binNeg, bu2, ident, mOut, zOut,
              t_steps, tb, sweeps)

    if compile:
        nc.compile()
    return nc


def _body(tc, nc, xT, winT, wrecT, uzT, binNeg, bu2, ident, mOut, zOut,
          t_steps, tb, sweeps):
    from contextlib import ExitStack

    nt = t_steps // tb          # number of GEMM tiles (halves)
    tw = tb * BL                # moving columns per tile (<=512)
    T1 = t_steps + 1            # state slots (slot j = value at step j-1)
    sp = 4 if nt % 4 == 0 else (2 if nt % 2 == 0 else 1)
    ng = nt // sp               # scan groups per sweep
    sb = sp * tb                # t-steps per scan group
    sw_cols = sp * tw           # columns per scan group
    pw = min(2, sp)             # tile-halves per psum tile
    PW = pw * tw                # psum tile columns (<=1024)
    npair = sp // pw            # psum tiles ("pairs") per group per cm
    pt = pw * tb                # t-steps per pair

    with ExitStack() as ctx:
        cpool = ctx.enter_context(tc.tile_pool(name="consts", bufs=1))
        xpool = ctx.enter_context(tc.tile_pool(name="xin", bufs=2))
        psbanks = (PW * 4 + 2047) // 2048
        pspool = ctx.enter_context(
            tc.tile_pool(name="ps", bufs=max(2, 8 // psbanks), space="PSUM"))
        upool = ctx.enter_context(tc.tile_pool(name="u", bufs=6))

        # ---- constants ----
        w_in = cpool.tile([128, KC * H], f16, tag="w_in")
        w_rec = cpool.tile([128, HC * H], f16, tag="w_rec")
        u_z = cpool.tile([128, HC * H], f16, tag="u_z")
        # interleave w_in and x-tile-0 per k-chunk so the first z-GEMM's
        # k-th matmul waits only on the k-th pair of transfers
        xs0 = xpool.tile([128, KC * tb * BL], f16, tag="xs", name="xs0")
        xw = tb * BL
        for k in range(KC):
            nc.sync.dma_start(w_in[:, k * H:(k + 1) * H], winT[k])
            nc.sync.dma_start(xs0[:, k * xw:(k + 1) * xw],
                              xT.ap()[0, :, k * xw:(k + 1) * xw])
        # x tile 1 next — ahead of the phase-1b/2 weights so the second
        # z-GEMM isn't queued behind them either
        xs1 = xpool.tile([128, KC * tb * BL], f16, tag="xs", name="xs1")
        if nt > 1:
            nc.sync.dma_start(xs1[:], xT[1])
        binS = cpool.tile([128, HC], f32, tag="binS")
        nc.sync.dma_start(binS[:], binNeg.ap().rearrange("c p -> p c"))
        buS = cpool.tile([128, HC], f32, tag="buS")
        nc.sync.dma_start(buS[:], bu2.ap().rearrange("c p -> p c"))
        for k in range(KC):
            nc.sync.dma_start(w_rec[:, k * H:(k + 1) * H], wrecT[k])
            nc.sync.dma_start(u_z[:, k * H:(k + 1) * H], uzT[k])
        idn = cpool.tile([128, 128], f16, tag="idn")
        nc.sync.dma_start(idn[:], ident[:])
        zzero = cpool.tile([128, 1], f32, tag="zzero")
        nc.vector.memset(zzero[:], 0.0)

        # ---- persistent whole-T tensors (T1 slot layout, slot 0 == 0) ----
        zneg = cpool.tile([128, HC * T1 * BL], f16, tag="zneg")   # -z
        ctil = cpool.tile([128, HC * t_steps * BL], f16, tag="ctil")
        dzb = cpool.tile([128, HC * t_steps * BL], f16, tag="dzb")
        mA = cpool.tile([128, KC * T1 * BL], f16, tag="mA")
        mB = cpool.tile([128, KC * T1 * BL], f16, tag="mB")
        m_bufs = [mA, mB]

        zn4 = zneg[:].rearrange("p (c b t) -> p c b t", c=HC, b=BL)
        dz4 = dzb[:].rearrange("p (c b t) -> p c b t", c=HC, b=BL)
        ct4 = ctil[:].rearrange("p (c b t) -> p c b t", c=HC, b=BL)
        mv4 = [m[:].rearrange("p (k b t) -> p k b t", k=KC, b=BL)
               for m in m_bufs]

        nc.vector.memset(zn4[:, :, :, 0], 0.0)      # z_{-1} = 0
        for mv in mv4:
            nc.vector.memset(mv[:, :, :, 0], 0.0)   # m_{-1} = 0

        # moving-operand slice of the T1 slot layout: [b][nsteps] cols
        def zslot(c, j0, nsteps):
            return zn4[:, c, :, j0:j0 + nsteps]

        # [h][b][t_local] view matching a psum pair's column order
        def hbt(t4, c, pts, hsteps, nh):
            v = t4[:, c, :, pts:pts + nh * hsteps]
            return v.rearrange("p b (h t) -> p h b t", h=nh)

        def scans(gi, gs, wv, uts, dma_fn=None, split=1):
            """Scans over one group: m_t = (dz_t + m_{t-1}) * u_t.

            uts: per-cm u tiles [128, sw_cols] laid out [b][t].  split>1
            chops the group into split chained sub-scans (tail-latency
            trim for the very last group), calling dma_fn after each.
            """
            step = sb // split
            for qi in range(split):
                qs = gs + qi * step
                for cm in range(HC):
                    u3 = uts[cm][:].rearrange("p (b t) -> p b t", b=BL)
                    for b in range(BL):
                        init = (zzero[:, 0:1] if (gi == 0 and qi == 0)
                                else wv[:, cm, b, qs:qs + 1])
                        nc.vector.tensor_tensor_scan(
                            wv[:, cm, b, 1 + qs:1 + qs + step],
                            dz4[:, cm, b, qs:qs + step],
                            u3[:, b, qs - gs:qs - gs + step],
                            init, op0=OP.add, op1=OP.mult)
                if dma_fn is not None:
                    dma_fn(qs, step)

        # ====== phase-1 group: z, dz, ct per pair; sweep-1 scans =========
        # zneg = -tanh(W_in x + b_in)
        # psum = U_z zneg_t + W_rec zneg_{t-1} = -(U_z z_t + W_rec z_{t-1})
        # ct   = -psum + b_u;  u^1 = sigmoid(ct)   (m^0 = 0)
        def phase1_group(gi):
            gs = gi * sb
            uts = [upool.tile([128, sw_cols], f16, tag="u", name=f"u1g{gi}c{cm}")
                   for cm in range(HC)]
            for pr in range(npair):
                pts = gs + pr * pt
                xs_h = []
                for h in range(pw):
                    ti = gi * sp + pr * pw + h
                    if ti == 0:
                        xs = xs0
                    elif ti == 1:
                        xs = xs1
                    else:
                        xs = xpool.tile([128, KC * tw], f16, tag="xs")
                        nc.sync.dma_start(xs[:], xT[ti])
                    xs_h.append(xs)
                for cm in range(HC):
                    ps = pspool.tile([128, PW], f32, tag="ps", name=f"za{cm}")
                    for h in range(pw):
                        for k in range(KC):
                            nc.tensor.matmul(
                                ps[:, h * tw:(h + 1) * tw],
                                w_in[:, k * H + cm * 128:k * H + cm * 128 + 128],
                                xs_h[h][:, k * tw:(k + 1) * tw],
                                start=(k == 0), stop=(k == KC - 1),
                                skip_group_check=True)
                    nc.scalar.activation(hbt(zn4, cm, 1 + pts, tb, pw), ps[:],
                                         AF.Tanh, bias=binS[:, cm:cm + 1],
                                         scale=-1.0)
                # dz_t = z_{t-1} - z_t = zneg_t - zneg_{t-1}  (DVE)
                nc.vector.tensor_sub(dz4[:, :, :, pts:pts + pt],
                                     zn4[:, :, :, 1 + pts:1 + pts + pt],
                                     zn4[:, :, :, pts:pts + pt])
                # stream z out for the host-side final s = z + m
                nc.sync.dma_start(zOut.ap()[:, :, :, 1 + pts:1 + pts + pt],
                                  zn4[:, :, :, 1 + pts:1 + pts + pt])
                for cm in range(HC):
                    ps = pspool.tile([128, PW], f32, tag="ps", name=f"cb{cm}")
                    for h in range(pw):
                        ts = pts + h * tb
                        hs = slice(h * tw, (h + 1) * tw)
                        for k in range(HC):
                            nc.tensor.matmul(
                                ps[:, hs],
                                u_z[:, k * H + cm * 128:k * H + cm * 128 + 128],
                                zslot(k, 1 + ts, tb),
                                start=(k == 0), stop=False,
                                skip_group_check=True)
                        for k in range(HC):
                            nc.tensor.matmul(
                                ps[:, hs],
                                w_rec[:, k * H + cm * 128:k * H + cm * 128 + 128],
                                zslot(k, ts, tb),
                                start=False, stop=(k == HC - 1),
                                skip_group_check=True)
                    nc.scalar.activation(
                        hbt(ct4, cm, pts, tb, pw), ps[:],
                        AF.Identity, bias=buS[:, cm:cm + 1], scale=-1.0)
                    nc.scalar.activation(
                        uts[cm][:].rearrange("p (b t) -> p b t", b=BL)
                        [:, :, pr * pt:pr * pt + pt]
                        .rearrange("p b (h t) -> p h b t", h=pw), ps[:],
                        AF.Sigmoid, bias=buS[:, cm:cm + 1], scale=-1.0)
            scans(gi, gs, mv4[1], uts)

        # ====== GEMM-sweep group (sweep index k in 2..sweeps) ============
        def sweep_group(ksw, gi):
            sw = ksw - 1
            rv = mv4[sw % 2]
            wv = mv4[(sw + 1) % 2]
            last = sw == sweeps - 1
            gs = gi * sb
            uts = [upool.tile([128, sw_cols], f16, tag="u",
                              name=f"u{ksw}g{gi}c{cm}")
                   for cm in range(HC)]
            for pr in range(npair):
                pts = gs + pr * pt
                pss = [pspool.tile([128, PW], f32, tag="ps", name=f"pp{cm}")
                       for cm in range(HC)]
                for cm in range(HC):
                    # inject ct into psum: cm<3 via ACT psum-prefill (Copy),
                    # cm=3 via identity matmuls — balances PE vs ACT load
                    if cm < 3:
                        nc.scalar.activation(
                            pss[cm][:], hbt(ct4, cm, pts, tb, pw), AF.Copy)
                    else:
                        for h in range(pw):
                            ts = pts + h * tb
                            nc.tensor.matmul(
                                pss[cm][:, h * tw:(h + 1) * tw], idn[:],
                                ct4[:, cm, :, ts:ts + tb],
                                start=True, stop=False, skip_group_check=True)
                for h in range(pw):
                    ts = pts + h * tb
                    for k in range(KC):
                        for cm in range(HC):
                            nc.tensor.matmul(
                                pss[cm][:, h * tw:(h + 1) * tw],
                                w_rec[:, k * H + cm * 128:
                                      k * H + cm * 128 + 128],
                                rv[:, k, :, ts:ts + tb],
                                start=False, stop=(k == KC - 1),
                                skip_group_check=True)
                for cm in range(HC):
                    nc.scalar.activation(
                        uts[cm][:].rearrange("p (b t) -> p b t", b=BL)
                        [:, :, pr * pt:pr * pt + pt]
                        .rearrange("p b (h t) -> p h b t", h=pw),
                        pss[cm][:], AF.Sigmoid)
            if last:
                def mdma(qs, ww):
                    for k in range(KC):
                        nc.sync.dma_start(
                            mOut.ap()[:, k, :, 1 + qs:1 + qs + ww],
                            wv[:, k, :, 1 + qs:1 + qs + ww])
                split = 2 if (gi == ng - 1 and sp >= 2) else 1
                scans(gi, gs, wv, uts, dma_fn=mdma, split=split)
            else:
                scans(gi, gs, wv, uts)

        # ====== wavefront interleave =====================================
        # Emit (sweep k, group g) at wave g + k - 1: sweep-2/3 scans fill
        # the DVE idle windows while phase-1's GEMM-heavy groups run.
        for w in range(ng + sweeps - 1):
            for ksw in range(1, sweeps + 1):
                g = w - (ksw - 1)
                if 0 <= g < ng:
                    if ksw == 1:
                        phase1_group(g)
                    else:
                        sweep_group(ksw, g)


_CACHED = {}


def _get_nc(t_steps=T, tb=TB):
    key = (t_steps, tb)
    if key not in _CACHED:
        _CACHED[key] = build(t_steps, tb)
    return _CACHED[key]


def make_in_maps(inputs, W_in, b_in, W_rec, U_z, b_u, t_steps=T):
    x = np.asarray(inputs, dtype=np.float32)
    winT_np = np.ascontiguousarray(
        np.asarray(W_in, np.float32).T.reshape(KC, 128, H)).astype(np.float16)
    wrecT_np = np.ascontiguousarray(
        np.asarray(W_rec, np.float32).T.reshape(HC, 128, H)).astype(np.float16)
    uzT_np = np.ascontiguousarray(
        np.asarray(U_z, np.float32).T.reshape(HC, 128, H)).astype(np.float16)
    binNeg_np = np.ascontiguousarray(
        (-np.asarray(b_in, np.float32)).reshape(HC, 128))
    bu_np = np.ascontiguousarray(np.asarray(b_u, np.float32).reshape(HC, 128))
    id_np = np.eye(128, dtype=np.float16)

    tb = min(TB, t_steps)
    nt = t_steps // tb
    in_maps = []
    for c in range(N_CORES):
        xc = x[c * BL:(c + 1) * BL, :t_steps, :]          # (BL, t, I)
        # -> [tile, 128, (k, b, t_local)] matching the SBUF tile layout
        xTc = np.ascontiguousarray(
            xc.reshape(BL, nt, tb, KC, 128).transpose(1, 4, 3, 0, 2)
        ).reshape(nt, 128, KC * tb * BL).astype(np.float16)
        in_maps.append({
            "xT": xTc, "winT": winT_np, "wrecT": wrecT_np, "uzT": uzT_np,
            "binNeg": binNeg_np, "bu2": bu_np, "ident": id_np,
        })
    return in_maps


def assemble_core(core_res, t_steps=T):
    """Host-side s = z + m from the device's m and zneg buffers."""
    m = np.asarray(core_res["mOut"])[:, :, :, 1:1 + t_steps]   # [128,KC,BL,T]
    zn = np.asarray(core_res["zOut"])[:, :, :, 1:1 + t_steps]  # [128,HC,BL,T]
    s = m.astype(np.float32) - zn.astype(np.float32)
    # [128, C, BL, T] -> [BL, T, C, 128] -> [BL, T, H]
    bl = s.shape[2]
    return np.ascontiguousarray(s.transpose(2, 3, 1, 0)).reshape(
        bl, t_steps, HC * 128)


def kernel(inputs, W_in, b_in, W_rec, U_z, b_u):
    nc = _get_nc()
    in_maps = make_in_maps(inputs, W_in, b_in, W_rec, U_z, b_u)
    res = bass_utils.run_bass_kernel_spmd(nc, in_maps, core_ids=list(range(N_CORES)))
    outs = [assemble_core(res.results[c]) for c in range(N_CORES)]
    return np.ascontiguousarray(np.concatenate(outs, axis=0), dtype=np.float32)


# revision 12
# speedup vs baseline: 1.2273x; 1.2273x over previous
"""MinRNN Trainium2 kernel — quasi-DEER fixed-point iteration, v6.

Model (per batch row):
    z_t = tanh(x_t @ W_in^T + b_in)
    u_t = sigmoid(s_{t-1} @ W_rec^T + z_t @ U_z^T + b_u)
    s_t = u_t * s_{t-1} + (1 - u_t) * z_t

Reformulated on the deviation m = s - z (with the convention z_{-1} = 0,
so m_{-1} = 0):

    pre_t = W_rec m_{t-1} + ct_t,   ct_t = U_z z_t + W_rec z_{t-1} + b_u
    u_t   = sigmoid(pre_t)
    m_t   = (dz_t + m_{t-1}) * u_t,  dz_t = z_{t-1} - z_t
    s_t   = z_t + m_t

Solved by fixed-point sweeps (quasi-DEER): freeze u from the previous
iterate, then the m-recurrence is solved EXACTLY by the HW linear-scan
instruction (tensor_tensor_scan computes state=(d0+state)*d1 in fp32).
K=4 effective sweeps (first is GEMM-free since m^0=0) converge to
rel err ~6.8e-3, 3x inside the 2e-2 tolerance.

v6 structural changes over v5 (225.4us):
  - scan groups sb=1024 steps (sp=4 tiles): 16 full-width scans per
    sweep instead of 32 — halves the ~0.7us/instr DVE fixed cost.
  - b-major (t innermost) layout for z/dz/ct/m/u: every scan operand
    is stride-1 (stride-2 APs ran the scans at ~1.5 cycles/elem).
  - psum tiles [128, 1024] (2 banks): ACT instructions run 1024 wide,
    halving the ~0.29us/instr ACT fixed cost (264 -> 132 ACTIVATEs).
  - dz on the Vector engine instead of GpSimd: the GpSimd SBUF port is
    an exclusive lock shared with DVE, so pool work stalled the scans.
  - u stored in per-cm [128, 2048] tiles (6 bufs) to keep SBUF at the
    v5 footprint.
Data-parallel over batch: 8 cores x 2 rows; fp16 GEMMs, fp32 psum.
GEMM moving columns are ordered [b][t] to match.
"""

import numpy as np
import ml_dtypes

import concourse.bass as bass
import concourse.mybir as mybir
import concourse.tile as tile
import concourse.bacc as bacc
from concourse import bass_utils

AF = mybir.ActivationFunctionType
OP = mybir.AluOpType

B, T, I, H = 16, 2048, 512, 512
N_CORES = 8
BL = B // N_CORES          # batch rows per core (2)
KC = I // 128              # input-dim chunks (4)
HC = H // 128              # hidden-dim chunks (4)
TB = 256                   # t-steps per GEMM tile (512 moving columns)
K_SWEEPS = 4               # effective sweeps (first one is GEMM-free)

f32 = mybir.dt.float32
f16 = mybir.dt.float16


def build(t_steps: int = T, tb: int = TB, sweeps: int = K_SWEEPS,
          compile: bool = True):
    tb = min(tb, t_steps)
    assert t_steps % tb == 0
    T1 = t_steps + 1

    nc = bacc.Bacc("TRN2", target_bir_lowering=False, debug=False)

    # x pre-tiled on the host: [tile, 128, (k t b)] so each tile DMA is a
    # fully contiguous [128, KC*tb*BL] block (4KB runs per partition)
    xT = nc.dram_tensor("xT", [t_steps // tb, 128, KC * tb * BL], f16,
                        kind="ExternalInput")
    winT = nc.dram_tensor("winT", [KC, 128, H], f16, kind="ExternalInput")
    wrecT = nc.dram_tensor("wrecT", [HC, 128, H], f16, kind="ExternalInput")
    uzT = nc.dram_tensor("uzT", [HC, 128, H], f16, kind="ExternalInput")
    binNeg = nc.dram_tensor("binNeg", [HC, 128], f32, kind="ExternalInput")
    bu2 = nc.dram_tensor("bu2", [HC, 128], f32, kind="ExternalInput")
    ident = nc.dram_tensor("ident", [128, 128], f16, kind="ExternalInput")
    mOut = nc.dram_tensor("mOut", [128, KC, BL, T1], f16, kind="ExternalOutput")
    zOut = nc.dram_tensor("zOut", [128, HC, BL, T1], f16, kind="ExternalOutput")

    with tile.TileContext(nc) as tc:
        _body(tc, nc, xT, winT, wrecT, uzT, binNeg, bu2, ident, mOut, zOut,
              t_steps, tb, sweeps)

    if compile:
        nc.compile()
    return nc


def _body(tc, nc, xT, winT, wrecT, uzT, binNeg, bu2, ident, mOut, zOut,
          t_steps, tb, sweeps):
    from contextlib import ExitStack

    nt = t_steps // tb          # number of GEMM tiles (halves)
    tw = tb * BL                # moving columns per tile (<=512)
    T1 = t_steps + 1            # state slots (slot j = value at step j-1)
    sp = 4 if nt % 4 == 0 else (2 if nt % 2 == 0 else 1)
    ng = nt // sp               # scan groups per sweep
    sb = sp * tb                # t-steps per scan group
    sw_cols = sp * tw           # columns per scan group
    pw = min(2, sp)             # tile-halves per psum tile
    PW = pw * tw                # psum tile columns (<=1024)
    npair = sp // pw            # psum tiles ("pairs") per group per cm
    pt = pw * tb                # t-steps per pair

    with ExitStack() as ctx:
        cpool = ctx.enter_context(tc.tile_pool(name="consts", bufs=1))
        xpool = ctx.enter_context(tc.tile_pool(name="xin", bufs=2))
        psbanks = (PW * 4 + 2047) // 2048
        pspool = ctx.enter_context(
            tc.tile_pool(name="ps", bufs=max(2, 8 // psbanks), space="PSUM"))
        upool = ctx.enter_context(tc.tile_pool(name="u", bufs=6))

        # ---- constants ----
        w_in = cpool.tile([128, KC * H], f16, tag="w_in")
        w_rec = cpool.tile([128, HC * H], f16, tag="w_rec")
        u_z = cpool.tile([128, HC * H], f16, tag="u_z")
        # interleave w_in and x-tile-0 per k-chunk so the first z-GEMM's
        # k-th matmul waits only on the k-th pair of transfers
        xs0 = xpool.tile([128, KC * tb * BL], f16, tag="xs", name="xs0")
        xw = tb * BL
        for k in range(KC):
            nc.sync.dma_start(w_in[:, k * H:(k + 1) * H], winT[k])
            nc.sync.dma_start(xs0[:, k * xw:(k + 1) * xw],
                              xT.ap()[0, :, k * xw:(k + 1) * xw])
        # x tile 1 next — ahead of the phase-1b/2 weights so the second
        # z-GEMM isn't queued behind them either
        xs1 = xpool.tile([128, KC * tb * BL], f16, tag="xs", name="xs1")
        if nt > 1:
            nc.sync.dma_start(xs1[:], xT[1])
        binS = cpool.tile([128, HC], f32, tag="binS")
        nc.sync.dma_start(binS[:], binNeg.ap().rearrange("c p -> p c"))
        buS = cpool.tile([128, HC], f32, tag="buS")
        nc.sync.dma_start(buS[:], bu2.ap().rearrange("c p -> p c"))
        for k in range(KC):
            nc.sync.dma_start(w_rec[:, k * H:(k + 1) * H], wrecT[k])
            nc.sync.dma_start(u_z[:, k * H:(k + 1) * H], uzT[k])
        idn = cpool.tile([128, 128], f16, tag="idn")
        nc.sync.dma_start(idn[:], ident[:])
        zzero = cpool.tile([128, 1], f32, tag="zzero")
        nc.vector.memset(zzero[:], 0.0)

        # ---- persistent whole-T tensors (T1 slot layout, slot 0 == 0) ----
        zneg = cpool.tile([128, HC * T1 * BL], f16, tag="zneg")   # -z
        ctil = cpool.tile([128, HC * t_steps * BL], f16, tag="ctil")
        dzb = cpool.tile([128, HC * t_steps * BL], f16, tag="dzb")
        mA = cpool.tile([128, KC * T1 * BL], f16, tag="mA")
        mB = cpool.tile([128, KC * T1 * BL], f16, tag="mB")
        m_bufs = [mA, mB]

        zn4 = zneg[:].rearrange("p (c b t) -> p c b t", c=HC, b=BL)
        dz4 = dzb[:].rearrange("p (c b t) -> p c b t", c=HC, b=BL)
        ct4 = ctil[:].rearrange("p (c b t) -> p c b t", c=HC, b=BL)
        mv4 = [m[:].rearrange("p (k b t) -> p k b t", k=KC, b=BL)
               for m in m_bufs]

        nc.vector.memset(zn4[:, :, :, 0], 0.0)      # z_{-1} = 0
        for mv in mv4:
            nc.vector.memset(mv[:, :, :, 0], 0.0)   # m_{-1} = 0

        # moving-operand slice of the T1 slot layout: [b][nsteps] cols
        def zslot(c, j0, nsteps):
            return zn4[:, c, :, j0:j0 + nsteps]

        # [h][b][t_local] view matching a psum pair's column order
        def hbt(t4, c, pts, hsteps, nh):
            v = t4[:, c, :, pts:pts + nh * hsteps]
            return v.rearrange("p b (h t) -> p h b t", h=nh)

        def scans(gi, gs, wv, uts, dma_fn=None, split=1):
            """Scans over one group: m_t = (dz_t + m_{t-1}) * u_t.

            uts: per-cm u tiles [128, sw_cols] laid out [b][t].  split>1
            chops the group into split chained sub-scans (tail-latency
            trim for the very last group), calling dma_fn after each.
            """
            step = sb // split
            for qi in range(split):
                qs = gs + qi * step
                for cm in range(HC):
                    u3 = uts[cm][:].rearrange("p (b t) -> p b t", b=BL)
                    for b in range(BL):
                        init = (zzero[:, 0:1] if (gi == 0 and qi == 0)
                                else wv[:, cm, b, qs:qs + 1])
                        nc.vector.tensor_tensor_scan(
                            wv[:, cm, b, 1 + qs:1 + qs + step],
                            dz4[:, cm, b, qs:qs + step],
                            u3[:, b, qs - gs:qs - gs + step],
                            init, op0=OP.add, op1=OP.mult)
                    if dma_fn is not None:
                        dma_fn(cm, qs, step)

        # ====== phase-1 group: z, dz, ct per pair; sweep-1 scans =========
        # zneg = -tanh(W_in x + b_in)
        # psum = U_z zneg_t + W_rec zneg_{t-1} = -(U_z z_t + W_rec z_{t-1})
        # ct   = -psum + b_u;  u^1 = sigmoid(ct)   (m^0 = 0)
        def phase1_group(gi):
            gs = gi * sb
            uts = [upool.tile([128, sw_cols], f16, tag="u", name=f"u1g{gi}c{cm}")
                   for cm in range(HC)]
            for pr in range(npair):
                pts = gs + pr * pt
                xs_h = []
                for h in range(pw):
                    ti = gi * sp + pr * pw + h
                    if ti == 0:
                        xs = xs0
                    elif ti == 1:
                        xs = xs1
                    else:
                        xs = xpool.tile([128, KC * tw], f16, tag="xs")
                        nc.sync.dma_start(xs[:], xT[ti])
                    xs_h.append(xs)
                for cm in range(HC):
                    ps = pspool.tile([128, PW], f32, tag="ps", name=f"za{cm}")
                    for h in range(pw):
                        for k in range(KC):
                            nc.tensor.matmul(
                                ps[:, h * tw:(h + 1) * tw],
                                w_in[:, k * H + cm * 128:k * H + cm * 128 + 128],
                                xs_h[h][:, k * tw:(k + 1) * tw],
                                start=(k == 0), stop=(k == KC - 1),
                                skip_group_check=True)
                    nc.scalar.activation(hbt(zn4, cm, 1 + pts, tb, pw), ps[:],
                                         AF.Tanh, bias=binS[:, cm:cm + 1],
                                         scale=-1.0)
                # dz_t = z_{t-1} - z_t = zneg_t - zneg_{t-1}  (DVE)
                nc.vector.tensor_sub(dz4[:, :, :, pts:pts + pt],
                                     zn4[:, :, :, 1 + pts:1 + pts + pt],
                                     zn4[:, :, :, pts:pts + pt])
                # stream z out for the host-side final s = z + m
                nc.sync.dma_start(zOut.ap()[:, :, :, 1 + pts:1 + pts + pt],
                                  zn4[:, :, :, 1 + pts:1 + pts + pt])
                for cm in range(HC):
                    ps = pspool.tile([128, PW], f32, tag="ps", name=f"cb{cm}")
                    for h in range(pw):
                        ts = pts + h * tb
                        hs = slice(h * tw, (h + 1) * tw)
                        for k in range(HC):
                            nc.tensor.matmul(
                                ps[:, hs],
                                u_z[:, k * H + cm * 128:k * H + cm * 128 + 128],
                                zslot(k, 1 + ts, tb),
                                start=(k == 0), stop=False,
                                skip_group_check=True)
                        for k in range(HC):
                            nc.tensor.matmul(
                                ps[:, hs],
                                w_rec[:, k * H + cm * 128:k * H + cm * 128 + 128],
                                zslot(k, ts, tb),
                                start=False, stop=(k == HC - 1),
                                skip_group_check=True)
                    nc.scalar.activation(
                        hbt(ct4, cm, pts, tb, pw), ps[:],
                        AF.Identity, bias=buS[:, cm:cm + 1], scale=-1.0)
                    nc.scalar.activation(
                        uts[cm][:].rearrange("p (b t) -> p b t", b=BL)
                        [:, :, pr * pt:pr * pt + pt]
                        .rearrange("p b (h t) -> p h b t", h=pw), ps[:],
                        AF.Sigmoid, bias=buS[:, cm:cm + 1], scale=-1.0)
            scans(gi, gs, mv4[1], uts)

        # ====== GEMM-sweep group (sweep index k in 2..sweeps) ============
        def sweep_group(ksw, gi):
            sw = ksw - 1
            rv = mv4[sw % 2]
            wv = mv4[(sw + 1) % 2]
            last = sw == sweeps - 1
            gs = gi * sb
            uts = [upool.tile([128, sw_cols], f16, tag="u",
                              name=f"u{ksw}g{gi}c{cm}")
                   for cm in range(HC)]
            for pr in range(npair):
                pts = gs + pr * pt
                # cm-outer so σ(cm) fires as soon as cm's psum drains and
                # the scans overlap the remaining cm GEMMs
                for cm in range(HC):
                    ps = pspool.tile([128, PW], f32, tag="ps", name=f"pp{cm}")
                    # inject ct into psum: cm<3 via ACT psum-prefill (Copy),
                    # cm=3 via identity matmuls — balances PE vs ACT load
                    if cm < 3:
                        nc.scalar.activation(
                            ps[:], hbt(ct4, cm, pts, tb, pw), AF.Copy)
                    else:
                        for h in range(pw):
                            ts = pts + h * tb
                            nc.tensor.matmul(
                                ps[:, h * tw:(h + 1) * tw], idn[:],
                                ct4[:, cm, :, ts:ts + tb],
                                start=True, stop=False, skip_group_check=True)
                    for h in range(pw):
                        ts = pts + h * tb
                        for k in range(KC):
                            nc.tensor.matmul(
                                ps[:, h * tw:(h + 1) * tw],
                                w_rec[:, k * H + cm * 128:
                                      k * H + cm * 128 + 128],
                                rv[:, k, :, ts:ts + tb],
                                start=False, stop=(k == KC - 1),
                                skip_group_check=True)
                    nc.scalar.activation(
                        uts[cm][:].rearrange("p (b t) -> p b t", b=BL)
                        [:, :, pr * pt:pr * pt + pt]
                        .rearrange("p b (h t) -> p h b t", h=pw),
                        ps[:], AF.Sigmoid)
            if last:
                def mdma(k, qs, ww):
                    nc.sync.dma_start(
                        mOut.ap()[:, k, :, 1 + qs:1 + qs + ww],
                        wv[:, k, :, 1 + qs:1 + qs + ww])
                split = 2 if (gi == ng - 1 and sp >= 2) else 1
                scans(gi, gs, wv, uts, dma_fn=mdma, split=split)
            else:
                scans(gi, gs, wv, uts)

        # ====== wavefront interleave =====================================
        # Emit (sweep k, group g) at wave g + k - 1: sweep-2/3 scans fill
        # the DVE idle windows while phase-1's GEMM-heavy groups run.
        for w in range(ng + sweeps - 1):
            for ksw in range(1, sweeps + 1):
                g = w - (ksw - 1)
                if 0 <= g < ng:
                    if ksw == 1:
                        phase1_group(g)
                    else:
                        sweep_group(ksw, g)


_CACHED = {}


def _get_nc(t_steps=T, tb=TB):
    key = (t_steps, tb)
    if key not in _CACHED:
        _CACHED[key] = build(t_steps, tb)
    return _CACHED[key]


def make_in_maps(inputs, W_in, b_in, W_rec, U_z, b_u, t_steps=T):
    x = np.asarray(inputs, dtype=np.float32)
    winT_np = np.ascontiguousarray(
        np.asarray(W_in, np.float32).T.reshape(KC, 128, H)).astype(np.float16)
    wrecT_np = np.ascontiguousarray(
        np.asarray(W_rec, np.float32).T.reshape(HC, 128, H)).astype(np.float16)
    uzT_np = np.ascontiguousarray(
        np.asarray(U_z, np.float32).T.reshape(HC, 128, H)).astype(np.float16)
    binNeg_np = np.ascontiguousarray(
        (-np.asarray(b_in, np.float32)).reshape(HC, 128))
    bu_np = np.ascontiguousarray(np.asarray(b_u, np.float32).reshape(HC, 128))
    id_np = np.eye(128, dtype=np.float16)

    tb = min(TB, t_steps)
    nt = t_steps // tb
    in_maps = []
    for c in range(N_CORES):
        xc = x[c * BL:(c + 1) * BL, :t_steps, :]          # (BL, t, I)
        # -> [tile, 128, (k, b, t_local)] matching the SBUF tile layout
        xTc = np.ascontiguousarray(
            xc.reshape(BL, nt, tb, KC, 128).transpose(1, 4, 3, 0, 2)
        ).reshape(nt, 128, KC * tb * BL).astype(np.float16)
        in_maps.append({
            "xT": xTc, "winT": winT_np, "wrecT": wrecT_np, "uzT": uzT_np,
            "binNeg": binNeg_np, "bu2": bu_np, "ident": id_np,
        })
    return in_maps


def assemble_core(core_res, t_steps=T):
    """Host-side s = z + m from the device's m and zneg buffers."""
    m = np.asarray(core_res["mOut"])[:, :, :, 1:1 + t_steps]   # [128,KC,BL,T]
    zn = np.asarray(core_res["zOut"])[:, :, :, 1:1 + t_steps]  # [128,HC,BL,T]
    s = m.astype(np.float32) - zn.astype(np.float32)
    # [128, C, BL, T] -> [BL, T, C, 128] -> [BL, T, H]
    bl = s.shape[2]
    return np.ascontiguousarray(s.transpose(2, 3, 1, 0)).reshape(
        bl, t_steps, HC * 128)


def kernel(inputs, W_in, b_in, W_rec, U_z, b_u):
    nc = _get_nc()
    in_maps = make_in_maps(inputs, W_in, b_in, W_rec, U_z, b_u)
    res = bass_utils.run_bass_kernel_spmd(nc, in_maps, core_ids=list(range(N_CORES)))
    outs = [assemble_core(res.results[c]) for c in range(N_CORES)]
    return np.ascontiguousarray(np.concatenate(outs, axis=0), dtype=np.float32)
